# revision 38
# baseline (speedup 1.0000x reference)
"""Multi-head attention Trainium2 Bass kernel.

Problem: B=4, S=2048, D=1024, H=16 heads (head_dim 64).
  q = (query @ Wq.T + bq).astype(f16); k, v likewise
  energy = einsum('bhqd,bhkd', q, k) / sqrt(64)   (f16)
  attn = softmax(energy, -1)                       (f16)
  x = einsum('bhqk,bhkd', attn, v).astype(f32)
  out = x @ Wo.T + bo                              (f32)

Sharding (8 cores): core c handles batch b = c//2 and head-group hg = c%2
(8 heads = 512 of the 1024 hidden dims).  QKV projections are column-split,
out-projection is row-split; the two partial outputs per batch are summed on
the host.  Biases: bq/bk are added on-chip (per-partition bias on the ACT
eviction); bv/bo contribute `bv_local @ WoT_local + bo` — a constant row
(softmax rows sum to 1) added on the host.

On-chip dataflow per core (all f16 matmul inputs, f32 PSUM):
  1. QT = WqT_loc.T @ XTq  -> [512, 2048] (d_local on partitions), same KT.
     V = XTv.T @ WvT_loc   -> [2048, 512] (s on partitions), stored per-head
     with an appended ones column (V_aug [128, 8*65]).
  2. Per head pair (row-tiled PE, head0 partitions 0:64 / head1 64:128) and
     q-half of 1024: for each k-chunk of 128:
       ST[k,q] scores (transposed layout), one ACT exp over [128, 2048]
       (scale=1/8 fused, no max subtraction - energies are ~N(0,1), exp fits
       f16 comfortably), AV matmul with ones-augmented V accumulating
       O_unnorm.T [65, 1024]; row 64 = softmax denominator.
  3. Normalize: reciprocal of denom row, gpsimd partition-broadcast,
     DVE multiply -> OT f16 (pair-packed [128, 2048] per d-chunk; odd head
     routed through a base-0 tmp tile + SBUF DMA to partitions 64:128).
  4. Out-projection: Y[q,1024] = sum_t OT_t.T @ WoT_t, f32 out.
"""

import os

import numpy as np

B, S, D, H = 4, 2048, 1024, 16
HD = 64
NCORES = 8
DL = 512  # d_local per core
HL = 8  # local heads per core
KC = 8  # contraction chunks (D / 128) for projections
DC = 4  # d_local chunks of 128
SC4 = 4  # S chunks of 512
SC16 = 16  # S chunks of 128
VW = HD + 1  # per-head V width incl. ones column (65)

_PROGRAM = None


def _build_program():
    import concourse.mybir as mybir
    import concourse.tile as tile
    from concourse import bacc

    f16 = mybir.dt.float16
    f32 = mybir.dt.float32
    ACT = mybir.ActivationFunctionType

    nc = bacc.Bacc("TRN2", target_bir_lowering=False, debug=False)

    xtq = nc.declare_dram_parameter("xtq", [D, S], f16, isOutput=False)
    xtk = nc.declare_dram_parameter("xtk", [D, S], f16, isOutput=False)
    xtv = nc.declare_dram_parameter("xtv", [D, S], f16, isOutput=False)
    wqt = nc.declare_dram_parameter("wqt", [D, DL], f16, isOutput=False)
    wkt = nc.declare_dram_parameter("wkt", [D, DL], f16, isOutput=False)
    wvt = nc.declare_dram_parameter("wvt", [D, DL], f16, isOutput=False)
    wot = nc.declare_dram_parameter("wot", [DL, D], f16, isOutput=False)
    bq = nc.declare_dram_parameter("bq", [DL], f32, isOutput=False)
    bk = nc.declare_dram_parameter("bk", [DL], f32, isOutput=False)
    y = nc.declare_dram_parameter("y", [S, D], f32, isOutput=True)

    with tile.TileContext(nc) as tc:
        # ---- persistent SBUF pools ----
        with (
            tc.tile_pool(name="wpool", bufs=1) as wpool,
            tc.tile_pool(name="bpool", bufs=1) as bpool,
            tc.tile_pool(name="qkv_sb", bufs=1) as qkv_sb,
            tc.tile_pool(name="ot_sb", bufs=1) as ot_pool,
        ):
            # weights: wx_sb[p, kc*512 + d] = WxT[kc*128 + p, d].  DMA order
            # matters (one HW queue): issue in first-use order — wv before
            # the xtv stream, wq/wk next, wo (used last) at the end.
            wq_sb = wpool.tile([128, KC * DL], f16, name="wq_sb")
            wk_sb = wpool.tile([128, KC * DL], f16, name="wk_sb")
            wv_sb = wpool.tile([128, KC * DL], f16, name="wv_sb")
            # wo_sb[p, t*1024 + o] = WoT[t*128 + p, o]
            wo_sb = wpool.tile([128, DC * D], f16, name="wo_sb")
            nc.sync.dma_start(
                wq_sb[:].rearrange("p (kc d) -> p kc d", d=DL),
                wqt.ap().rearrange("(kc p) d -> p kc d", p=128),
            )
            nc.sync.dma_start(
                wk_sb[:].rearrange("p (kc d) -> p kc d", d=DL),
                wkt.ap().rearrange("(kc p) d -> p kc d", p=128),
            )
            # biases as [128, DC] (per-partition scalars per d-chunk)
            bq_sb = bpool.tile([128, DC], f32, name="bq_sb")
            bk_sb = bpool.tile([128, DC], f32, name="bk_sb")
            nc.sync.dma_start(bq_sb[:], bq.ap().rearrange("(t p) -> p t", p=128))
            nc.sync.dma_start(bk_sb[:], bk.ap().rearrange("(t p) -> p t", p=128))

            # persistent activations
            qt_sb = [qkv_sb.tile([128, S], f16, name=f"qt{t}") for t in range(DC)]
            kt_sb = [qkv_sb.tile([128, S], f16, name=f"kt{t}") for t in range(DC)]
            v_sb = [qkv_sb.tile([128, HL * VW], f16, name=f"v{sc}") for sc in range(SC16)]
            ot_sb = [ot_pool.tile([128, S], f16, name=f"ot{t}") for t in range(DC)]

            # One PSUM pool for the whole kernel, three tags:
            #   stq: [128, 1024] f32, 2 slots (scores double-buffer)
            #   av:  [65, 512] f32, 2 slots (AV accumulator pair)
            #   ps:  [128, 512] f32, 2 slots (QKV + out-projection
            #        accumulators — never busy at the same time)
            # 2*2 + 2*1 + 2*1 = 8 banks.  Separate av/ps tags let attention
            # start while the K projection is still draining.
            with (
                tc.tile_pool(name="psum", bufs=1, space="PSUM") as psum,
                tc.tile_pool(name="xt_pool", bufs=16) as xt_pool,
                tc.tile_pool(name="vsl_pool", bufs=3) as vsl_pool,
                tc.tile_pool(name="e_pool", bufs=5) as e_pool,
                tc.tile_pool(name="n_pool", bufs=1) as n_pool,
                tc.tile_pool(name="y_pool", bufs=2) as y_pool,
            ):

                def load_xt(x_dram, engine=None):
                    # per-contraction-chunk tiles: xt_c[kc][p, s] = XT[kc*128+p, s]
                    engine = engine or nc.sync
                    xt_c = []
                    for kc in range(KC):
                        xt_t = xt_pool.tile([128, S], f16, name="xt", tag="xt")
                        engine.dma_start(
                            xt_t[:], x_dram.ap()[kc * 128 : (kc + 1) * 128, :]
                        )
                        xt_c.append(xt_t)
                    return xt_c

                def qk_group(xt_c, w_sb, out_tiles, b_ap, dc, sc):
                    # one QT/KT projection group: out [d_local(part), 512 s]
                    ps = psum.tile([128, 512], f32, name="ps", tag="ps", bufs=2)
                    for kc in range(KC):
                        nc.tensor.matmul(
                            ps[:],
                            lhsT=w_sb[:, kc * DL + dc * 128 : kc * DL + dc * 128 + 128],
                            rhs=xt_c[kc][:, sc * 512 : (sc + 1) * 512],
                            start=(kc == 0),
                            stop=(kc == KC - 1),
                        )
                    # eviction + per-partition bias on DVE (keeps ACT free
                    # for attention exp)
                    nc.vector.tensor_scalar_add(
                        out_tiles[dc][:, sc * 512 : (sc + 1) * 512],
                        ps[:],
                        b_ap[:, dc : dc + 1],
                    )

                def qk_proj(xt_c, w_sb, out_tiles, b_ap, dc):
                    for sc in range(SC4):
                        qk_group(xt_c, w_sb, out_tiles, b_ap, dc, sc)

                def v_proj(sc):
                    # own sliced input tile vt[p, kc*128+si] = XTv[kc*128+p,
                    # sc*128+si] — keeps xtv out of the xt slot ring (a
                    # shared ring deadlocks: xtv loads would wait on slots
                    # that only free after the last Q/K projection, which is
                    # scheduled after attention that needs V)
                    vt = vsl_pool.tile([128, KC * 128], f16, name="vt", tag="vt")
                    nc.sync.dma_start(
                        vt[:].rearrange("p (kc si) -> p kc si", si=128),
                        xtv.ap().rearrange("(kc p) s -> p kc s", p=128)[
                            :, :, sc * 128 : (sc + 1) * 128
                        ],
                    )
                    ps = psum.tile([128, 512], f32, name="ps", tag="ps", bufs=2)
                    for kc in range(KC):
                        nc.tensor.matmul(
                            ps[:],
                            lhsT=vt[:, kc * 128 : (kc + 1) * 128],
                            rhs=wv_sb[:, kc * DL : (kc + 1) * DL],
                            start=(kc == 0),
                            stop=(kc == KC - 1),
                        )
                    v3 = v_sb[sc][:].rearrange("p (h x) -> p h x", x=VW)
                    nc.vector.tensor_copy(
                        v3[:, :, 0:HD], ps[:].rearrange("p (h x) -> p h x", x=HD)
                    )
                    nc.vector.memset(v3[:, :, HD : HD + 1], 1.0)

                def attention(qq, pr, interleave=None):
                    q0 = qq * 512
                    h0, h1 = 2 * pr, 2 * pr + 1
                    av0 = psum.tile([VW, 512], f32, name="av", tag="av", bufs=2)
                    av1 = psum.tile([VW, 512], f32, name="av", tag="av", bufs=2)
                    for kc in range(SC16):
                        if interleave is not None:
                            interleave(kc)
                        st = psum.tile([128, 1024], f32, name="st", tag="stq", bufs=2)
                        nc.tensor.matmul(
                            st[:, 0:512],
                            lhsT=kt_sb[pr][0:64, kc * 128 : (kc + 1) * 128],
                            rhs=qt_sb[pr][0:64, q0 : q0 + 512],
                            start=True,
                            stop=True,
                        )
                        nc.tensor.matmul(
                            st[:, 512:1024],
                            lhsT=kt_sb[pr][64:128, kc * 128 : (kc + 1) * 128],
                            rhs=qt_sb[pr][64:128, q0 : q0 + 512],
                            start=True,
                            stop=True,
                        )
                        e = e_pool.tile([128, 1024], f16, name="e", tag="e")
                        nc.scalar.activation(e[:], st[:], ACT.Exp, scale=0.125)
                        nc.tensor.matmul(
                            av0[:],
                            lhsT=v_sb[kc][:, h0 * VW : (h0 + 1) * VW],
                            rhs=e[:, 0:512],
                            start=(kc == 0),
                            stop=(kc == SC16 - 1),
                            skip_group_check=True,
                        )
                        nc.tensor.matmul(
                            av1[:],
                            lhsT=v_sb[kc][:, h1 * VW : (h1 + 1) * VW],
                            rhs=e[:, 512:1024],
                            start=(kc == 0),
                            stop=(kc == SC16 - 1),
                            skip_group_check=True,
                        )
                    # Evict AV PSUM -> SBUF immediately (frees the banks for
                    # the next pair; normalization then runs out of SBUF off
                    # the PE critical path).
                    od0 = n_pool.tile([VW, 512], f32, name="od0", tag="od0")
                    od1 = n_pool.tile([VW, 512], f32, name="od1", tag="od1")
                    nc.vector.tensor_copy(od0[:], av0[:])
                    nc.vector.tensor_copy(od1[:], av1[:])
                    # normalize: O.T[hd, q] * (1 / denom[q]).  Denom rows sit
                    # at partition 64; DVE keeps partition bases aligned and
                    # partition_broadcast only reads base-0 offset-0 sources
                    # on HW: SBUF DMA down to p0 -> reciprocal -> gpsimd
                    # broadcast.
                    dd = n_pool.tile([1, 1024], f32, name="dd", tag="dd")
                    nc.sync.dma_start(dd[:, 0:512], od0[HD : HD + 1, :])
                    nc.sync.dma_start(dd[:, 512:1024], od1[HD : HD + 1, :])
                    r0 = n_pool.tile([1, 512], f32, name="r0", tag="r0")
                    r1 = n_pool.tile([1, 512], f32, name="r1", tag="r1")
                    nc.vector.reciprocal_approx_fast(r0[:], dd[:, 0:512])
                    nc.vector.reciprocal_approx_fast(r1[:], dd[:, 512:1024])
                    bc_a = n_pool.tile([64, 512], f32, name="bc_a", tag="bc_a")
                    bc_b = n_pool.tile([64, 512], f32, name="bc_b", tag="bc_b")
                    nc.gpsimd.partition_broadcast(bc_a[:], r0[:])
                    nc.gpsimd.partition_broadcast(bc_b[:], r1[:])
                    cols = slice(q0, q0 + 512)
                    nc.vector.tensor_mul(ot_sb[pr][0:64, cols], od0[0:64, :], bc_a[:])
                    tmp = n_pool.tile([64, 512], f16, name="tmp", tag="tmp")
                    nc.vector.tensor_mul(tmp[:], od1[0:64, :], bc_b[:])
                    nc.sync.dma_start(ot_sb[pr][64:128, cols], tmp[:])

                def out_proj(qq):
                    # 4 q-chunks of 128; 512-wide pieces on the ps tag so
                    # scores keep both stq slots
                    for mc in range(qq * 4, qq * 4 + 4):
                        yt = y_pool.tile([128, 1024], f32, name="yt", tag="yt")
                        for pc in range(2):
                            pso = psum.tile([128, 512], f32, name="pso", tag="ps", bufs=2)
                            for t in range(DC):
                                nc.tensor.matmul(
                                    pso[:],
                                    lhsT=ot_sb[t][:, mc * 128 : (mc + 1) * 128],
                                    rhs=wo_sb[:, t * D + pc * 512 : t * D + (pc + 1) * 512],
                                    start=(t == 0),
                                    stop=(t == DC - 1),
                                    skip_group_check=True,
                                )
                            nc.vector.tensor_copy(yt[:, pc * 512 : (pc + 1) * 512], pso[:])
                        nc.sync.dma_start(y.ap()[mc * 128 : (mc + 1) * 128, :], yt[:])

                # Input streams in first-use order on the one DMA queue:
                # xtq/xtk feed the first scores — interleaved by s-column so
                # the first projection groups start early; wv next (V input
                # slices are DMA'd per-group inside v_proj); wo last.
                xtq_c = load_xt(xtq)
                xtk_c = load_xt(xtk, engine=nc.scalar)
                nc.sync.dma_start(
                    wv_sb[:].rearrange("p (kc d) -> p kc d", d=DL),
                    wvt.ap().rearrange("(kc p) d -> p kc d", p=128),
                )
                nc.sync.dma_start(
                    wo_sb[:].rearrange("p (t o) -> p t o", o=D),
                    wot.ap().rearrange("(t p) o -> p t o", p=128),
                )

                # ---- pair-outer schedule.  Per pair: 4 ACT-bound attention
                # q-blocks; the NEXT pair's Q/K projections are emitted
                # after them (lower scheduler priority) so they fill the PE
                # idle inside the exp-paced stream.  attention(0,0) is
                # chunk-interleaved with the V projection (its first
                # consumer); out-projection rides the last pair's stretch.
                # first pair's projections interleaved q/k per s-column so
                # the first scores (needing q col-block 0 + k col-block 0)
                # start after two groups, not eight
                for sc in range(SC4):
                    qk_group(xtq_c, wq_sb, qt_sb, bq_sb, 0, sc)
                    qk_group(xtk_c, wk_sb, kt_sb, bk_sb, 0, sc)
                for pr in range(DC):
                    for qq in range(4):
                        inter = v_proj if (pr == 0 and qq == 0) else None
                        attention(qq, pr, interleave=inter)
                        if pr == DC - 1:
                            out_proj(qq)
                    if pr < DC - 1:
                        qk_proj(xtq_c, wq_sb, qt_sb, bq_sb, pr + 1)
                        qk_proj(xtk_c, wk_sb, kt_sb, bk_sb, pr + 1)

    nc.compile()
    return nc


def get_program():
    global _PROGRAM
    if _PROGRAM is None:
        _PROGRAM = _build_program()
    return _PROGRAM


def make_in_maps(query, key, value, Wq, bq, Wk, bk, Wv, bv, Wo, bo):
    """Per-core input dicts. Core c: batch c//2, head-group c%2."""
    query = np.asarray(query, np.float32)
    key = np.asarray(key, np.float32)
    value = np.asarray(value, np.float32)
    xt = {}
    for b in range(B):
        xt[b] = (
            np.ascontiguousarray(query[b].T.astype(np.float16)),
            np.ascontiguousarray(key[b].T.astype(np.float16)),
            np.ascontiguousarray(value[b].T.astype(np.float16)),
        )
    wslices = {}
    for hg in range(2):
        sl = slice(hg * DL, (hg + 1) * DL)
        wslices[hg] = dict(
            wqt=np.ascontiguousarray(np.asarray(Wq, np.float32)[sl, :].T.astype(np.float16)),
            wkt=np.ascontiguousarray(np.asarray(Wk, np.float32)[sl, :].T.astype(np.float16)),
            wvt=np.ascontiguousarray(np.asarray(Wv, np.float32)[sl, :].T.astype(np.float16)),
            wot=np.ascontiguousarray(np.asarray(Wo, np.float32)[:, sl].T.astype(np.float16)),
            bq=np.ascontiguousarray(np.asarray(bq, np.float32)[sl]),
            bk=np.ascontiguousarray(np.asarray(bk, np.float32)[sl]),
        )
    in_maps = []
    for c in range(NCORES):
        b, hg = c // 2, c % 2
        m = dict(xtq=xt[b][0], xtk=xt[b][1], xtv=xt[b][2])
        m.update(wslices[hg])
        in_maps.append(m)
    return in_maps


def combine_outputs(results, Wo, bo, bv):
    """Sum the two head-group partials per batch + host-side bias constant."""
    Wo = np.asarray(Wo, np.float32)
    bo = np.asarray(bo, np.float32)
    bv = np.asarray(bv, np.float32)
    const = bv @ Wo.T + bo  # [D]
    out = np.empty((B, S, D), np.float32)
    for b in range(B):
        out[b] = results[2 * b]["y"] + results[2 * b + 1]["y"] + const
    return out


def kernel(query, key, value, Wq, bq, Wk, bk, Wv, bv, Wo, bo):
    from concourse.bass_utils import run_bass_kernel_spmd

    nc = get_program()
    in_maps = make_in_maps(query, key, value, Wq, bq, Wk, bk, Wv, bv, Wo, bo)
    res = run_bass_kernel_spmd(nc, in_maps, core_ids=list(range(NCORES)))
    return combine_outputs(res.results, Wo, bo, bv)


# revision 42
# speedup vs baseline: 1.0224x; 1.0224x over previous
"""Multi-head attention Trainium2 Bass kernel.

Problem: B=4, S=2048, D=1024, H=16 heads (head_dim 64).
  q = (query @ Wq.T + bq).astype(f16); k, v likewise
  energy = einsum('bhqd,bhkd', q, k) / sqrt(64)   (f16)
  attn = softmax(energy, -1)                       (f16)
  x = einsum('bhqk,bhkd', attn, v).astype(f32)
  out = x @ Wo.T + bo                              (f32)

Sharding (8 cores): core c handles batch b = c//2 and head-group hg = c%2
(8 heads = 512 of the 1024 hidden dims).  QKV projections are column-split,
out-projection is row-split; the two partial outputs per batch are summed on
the host.  Biases: bq/bk are added on-chip (per-partition bias on the ACT
eviction); bv/bo contribute `bv_local @ WoT_local + bo` — a constant row
(softmax rows sum to 1) added on the host.

On-chip dataflow per core (all f16 matmul inputs, f32 PSUM):
  1. QT = WqT_loc.T @ XTq  -> [512, 2048] (d_local on partitions), same KT.
     V = XTv.T @ WvT_loc   -> [2048, 512] (s on partitions), stored per-head
     with an appended ones column (V_aug [128, 8*65]).
  2. Per head pair (row-tiled PE, head0 partitions 0:64 / head1 64:128) and
     q-half of 1024: for each k-chunk of 128:
       ST[k,q] scores (transposed layout), one ACT exp over [128, 2048]
       (scale=1/8 fused, no max subtraction - energies are ~N(0,1), exp fits
       f16 comfortably), AV matmul with ones-augmented V accumulating
       O_unnorm.T [65, 1024]; row 64 = softmax denominator.
  3. Normalize: reciprocal of denom row, gpsimd partition-broadcast,
     DVE multiply -> OT f16 (pair-packed [128, 2048] per d-chunk; odd head
     routed through a base-0 tmp tile + SBUF DMA to partitions 64:128).
  4. Out-projection: Y[q,1024] = sum_t OT_t.T @ WoT_t, f32 out.
"""

import os

import numpy as np

B, S, D, H = 4, 2048, 1024, 16
HD = 64
NCORES = 8
DL = 512  # d_local per core
HL = 8  # local heads per core
KC = 8  # contraction chunks (D / 128) for projections
DC = 4  # d_local chunks of 128
SC4 = 4  # S chunks of 512
SC16 = 16  # S chunks of 128
VW = HD + 1  # per-head V width incl. ones column (65)

_PROGRAM = None


def _build_program():
    import concourse.mybir as mybir
    import concourse.tile as tile
    from concourse import bacc

    f16 = mybir.dt.float16
    f32 = mybir.dt.float32
    ACT = mybir.ActivationFunctionType

    nc = bacc.Bacc("TRN2", target_bir_lowering=False, debug=False)

    xtq = nc.declare_dram_parameter("xtq", [D, S], f16, isOutput=False)
    xtk = nc.declare_dram_parameter("xtk", [D, S], f16, isOutput=False)
    xtv = nc.declare_dram_parameter("xtv", [D, S], f16, isOutput=False)
    wqt = nc.declare_dram_parameter("wqt", [D, DL], f16, isOutput=False)
    wkt = nc.declare_dram_parameter("wkt", [D, DL], f16, isOutput=False)
    wvt = nc.declare_dram_parameter("wvt", [D, DL], f16, isOutput=False)
    wot = nc.declare_dram_parameter("wot", [DL, D], f16, isOutput=False)
    bq = nc.declare_dram_parameter("bq", [DL], f32, isOutput=False)
    bk = nc.declare_dram_parameter("bk", [DL], f32, isOutput=False)
    y = nc.declare_dram_parameter("y", [S, D], f32, isOutput=True)

    with tile.TileContext(nc) as tc:
        # ---- persistent SBUF pools ----
        with (
            tc.tile_pool(name="wpool", bufs=1) as wpool,
            tc.tile_pool(name="bpool", bufs=1) as bpool,
            tc.tile_pool(name="qkv_sb", bufs=1) as qkv_sb,
            tc.tile_pool(name="ot_sb", bufs=1) as ot_pool,
        ):
            # weights: wx_sb[p, kc*512 + d] = WxT[kc*128 + p, d].  DMA order
            # matters (one HW queue): issue in first-use order — wv before
            # the xtv stream, wq/wk next, wo (used last) at the end.
            wq_sb = wpool.tile([128, KC * DL], f16, name="wq_sb")
            wk_sb = wpool.tile([128, KC * DL], f16, name="wk_sb")
            wv_sb = wpool.tile([128, KC * DL], f16, name="wv_sb")
            # wo_sb[p, t*1024 + o] = WoT[t*128 + p, o]
            wo_sb = wpool.tile([128, DC * D], f16, name="wo_sb")
            nc.sync.dma_start(
                wq_sb[:].rearrange("p (kc d) -> p kc d", d=DL),
                wqt.ap().rearrange("(kc p) d -> p kc d", p=128),
            )
            nc.sync.dma_start(
                wk_sb[:].rearrange("p (kc d) -> p kc d", d=DL),
                wkt.ap().rearrange("(kc p) d -> p kc d", p=128),
            )
            # biases as [128, DC] (per-partition scalars per d-chunk)
            bq_sb = bpool.tile([128, DC], f32, name="bq_sb")
            bk_sb = bpool.tile([128, DC], f32, name="bk_sb")
            nc.sync.dma_start(bq_sb[:], bq.ap().rearrange("(t p) -> p t", p=128))
            nc.sync.dma_start(bk_sb[:], bk.ap().rearrange("(t p) -> p t", p=128))

            # persistent activations
            qt_sb = [qkv_sb.tile([128, S], f16, name=f"qt{t}") for t in range(DC)]
            kt_sb = [qkv_sb.tile([128, S], f16, name=f"kt{t}") for t in range(DC)]
            v_sb = [qkv_sb.tile([128, HL * VW], f16, name=f"v{sc}") for sc in range(SC16)]
            ot_sb = [ot_pool.tile([128, S], f16, name=f"ot{t}") for t in range(DC)]

            # One PSUM pool for the whole kernel, three tags:
            #   stq: [128, 1024] f32, 2 slots (scores double-buffer)
            #   av:  [65, 512] f32, 2 slots (AV accumulator pair)
            #   ps:  [128, 512] f32, 2 slots (QKV + out-projection
            #        accumulators — never busy at the same time)
            # 2*2 + 2*1 + 2*1 = 8 banks.  Separate av/ps tags let attention
            # start while the K projection is still draining.
            with (
                tc.tile_pool(name="psum", bufs=1, space="PSUM") as psum,
                tc.tile_pool(name="xt_pool", bufs=16) as xt_pool,
                tc.tile_pool(name="vsl_pool", bufs=3) as vsl_pool,
                tc.tile_pool(name="e_pool", bufs=5) as e_pool,
                tc.tile_pool(name="n_pool", bufs=1) as n_pool,
                tc.tile_pool(name="y_pool", bufs=2) as y_pool,
            ):

                def load_xt(x_dram, engine=None):
                    # per-contraction-chunk tiles: xt_c[kc][p, s] = XT[kc*128+p, s]
                    engine = engine or nc.sync
                    xt_c = []
                    for kc in range(KC):
                        xt_t = xt_pool.tile([128, S], f16, name="xt", tag="xt")
                        engine.dma_start(
                            xt_t[:], x_dram.ap()[kc * 128 : (kc + 1) * 128, :]
                        )
                        xt_c.append(xt_t)
                    return xt_c

                def qk_group(xt_c, w_sb, out_tiles, b_ap, dc, sc):
                    # one QT/KT projection group: out [d_local(part), 512 s]
                    ps = psum.tile([128, 512], f32, name="ps", tag="ps", bufs=2)
                    for kc in range(KC):
                        nc.tensor.matmul(
                            ps[:],
                            lhsT=w_sb[:, kc * DL + dc * 128 : kc * DL + dc * 128 + 128],
                            rhs=xt_c[kc][:, sc * 512 : (sc + 1) * 512],
                            start=(kc == 0),
                            stop=(kc == KC - 1),
                        )
                    # eviction + per-partition bias on DVE (keeps ACT free
                    # for attention exp)
                    nc.vector.tensor_scalar_add(
                        out_tiles[dc][:, sc * 512 : (sc + 1) * 512],
                        ps[:],
                        b_ap[:, dc : dc + 1],
                    )

                def qk_proj(xt_c, w_sb, out_tiles, b_ap, dc):
                    for sc in range(SC4):
                        qk_group(xt_c, w_sb, out_tiles, b_ap, dc, sc)

                def v_proj(sc):
                    # own sliced input tile vt[p, kc*128+si] = XTv[kc*128+p,
                    # sc*128+si] — keeps xtv out of the xt slot ring (a
                    # shared ring deadlocks: xtv loads would wait on slots
                    # that only free after the last Q/K projection, which is
                    # scheduled after attention that needs V)
                    vt = vsl_pool.tile([128, KC * 128], f16, name="vt", tag="vt")
                    nc.sync.dma_start(
                        vt[:].rearrange("p (kc si) -> p kc si", si=128),
                        xtv.ap().rearrange("(kc p) s -> p kc s", p=128)[
                            :, :, sc * 128 : (sc + 1) * 128
                        ],
                    )
                    ps = psum.tile([128, 512], f32, name="ps", tag="ps", bufs=2)
                    for kc in range(KC):
                        nc.tensor.matmul(
                            ps[:],
                            lhsT=vt[:, kc * 128 : (kc + 1) * 128],
                            rhs=wv_sb[:, kc * DL : (kc + 1) * DL],
                            start=(kc == 0),
                            stop=(kc == KC - 1),
                        )
                    v3 = v_sb[sc][:].rearrange("p (h x) -> p h x", x=VW)
                    nc.vector.tensor_copy(
                        v3[:, :, 0:HD], ps[:].rearrange("p (h x) -> p h x", x=HD)
                    )
                    nc.vector.memset(v3[:, :, HD : HD + 1], 1.0)

                def attention(qq, pr, interleave=None, fillers=None):
                    q0 = qq * 512
                    h0, h1 = 2 * pr, 2 * pr + 1
                    av0 = psum.tile([VW, 512], f32, name="av", tag="av", bufs=2)
                    av1 = psum.tile([VW, 512], f32, name="av", tag="av", bufs=2)
                    for kc in range(SC16):
                        if interleave is not None:
                            interleave(kc)
                        if fillers:
                            fillers.pop(0)()
                        st = psum.tile([128, 1024], f32, name="st", tag="stq", bufs=2)
                        nc.tensor.matmul(
                            st[:, 0:512],
                            lhsT=kt_sb[pr][0:64, kc * 128 : (kc + 1) * 128],
                            rhs=qt_sb[pr][0:64, q0 : q0 + 512],
                            start=True,
                            stop=True,
                        )
                        nc.tensor.matmul(
                            st[:, 512:1024],
                            lhsT=kt_sb[pr][64:128, kc * 128 : (kc + 1) * 128],
                            rhs=qt_sb[pr][64:128, q0 : q0 + 512],
                            start=True,
                            stop=True,
                        )
                        e = e_pool.tile([128, 1024], f16, name="e", tag="e")
                        nc.scalar.activation(e[:], st[:], ACT.Exp, scale=0.125)
                        nc.tensor.matmul(
                            av0[:],
                            lhsT=v_sb[kc][:, h0 * VW : (h0 + 1) * VW],
                            rhs=e[:, 0:512],
                            start=(kc == 0),
                            stop=(kc == SC16 - 1),
                            skip_group_check=True,
                        )
                        nc.tensor.matmul(
                            av1[:],
                            lhsT=v_sb[kc][:, h1 * VW : (h1 + 1) * VW],
                            rhs=e[:, 512:1024],
                            start=(kc == 0),
                            stop=(kc == SC16 - 1),
                            skip_group_check=True,
                        )
                    # Evict AV PSUM -> SBUF immediately (frees the banks for
                    # the next pair; normalization then runs out of SBUF off
                    # the PE critical path).
                    od0 = n_pool.tile([VW, 512], f32, name="od0", tag="od0")
                    od1 = n_pool.tile([VW, 512], f32, name="od1", tag="od1")
                    nc.vector.tensor_copy(od0[:], av0[:])
                    nc.vector.tensor_copy(od1[:], av1[:])
                    # normalize: O.T[hd, q] * (1 / denom[q]).  Denom rows sit
                    # at partition 64; DVE keeps partition bases aligned and
                    # partition_broadcast only reads base-0 offset-0 sources
                    # on HW: SBUF DMA down to p0 -> reciprocal -> gpsimd
                    # broadcast.
                    dd = n_pool.tile([1, 1024], f32, name="dd", tag="dd")
                    nc.sync.dma_start(dd[:, 0:512], od0[HD : HD + 1, :])
                    nc.sync.dma_start(dd[:, 512:1024], od1[HD : HD + 1, :])
                    r0 = n_pool.tile([1, 512], f32, name="r0", tag="r0")
                    r1 = n_pool.tile([1, 512], f32, name="r1", tag="r1")
                    nc.vector.reciprocal_approx_fast(r0[:], dd[:, 0:512])
                    nc.vector.reciprocal_approx_fast(r1[:], dd[:, 512:1024])
                    bc_a = n_pool.tile([64, 512], f32, name="bc_a", tag="bc_a")
                    bc_b = n_pool.tile([64, 512], f32, name="bc_b", tag="bc_b")
                    nc.gpsimd.partition_broadcast(bc_a[:], r0[:])
                    nc.gpsimd.partition_broadcast(bc_b[:], r1[:])
                    cols = slice(q0, q0 + 512)
                    nc.vector.tensor_mul(ot_sb[pr][0:64, cols], od0[0:64, :], bc_a[:])
                    tmp = n_pool.tile([64, 512], f16, name="tmp", tag="tmp")
                    nc.vector.tensor_mul(tmp[:], od1[0:64, :], bc_b[:])
                    nc.sync.dma_start(ot_sb[pr][64:128, cols], tmp[:])

                def oproj_piece(mc, pc, state):
                    # one 512-wide accumulation group of the out-projection
                    # for q-chunk mc; pc==1 finishes the chunk (evict + DMA)
                    if pc == 0:
                        state[mc] = y_pool.tile([128, 1024], f32, name="yt", tag="yt")
                    yt = state[mc]
                    pso = psum.tile([128, 512], f32, name="pso", tag="ps", bufs=2)
                    for t in range(DC):
                        nc.tensor.matmul(
                            pso[:],
                            lhsT=ot_sb[t][:, mc * 128 : (mc + 1) * 128],
                            rhs=wo_sb[:, t * D + pc * 512 : t * D + (pc + 1) * 512],
                            start=(t == 0),
                            stop=(t == DC - 1),
                            skip_group_check=True,
                        )
                    nc.vector.tensor_copy(yt[:, pc * 512 : (pc + 1) * 512], pso[:])
                    if pc == 1:
                        nc.sync.dma_start(y.ap()[mc * 128 : (mc + 1) * 128, :], yt[:])

                def oproj_fillers(qq):
                    state = {}
                    return [
                        (lambda mc=mc, pc=pc: oproj_piece(mc, pc, state))
                        for mc in range(qq * 4, qq * 4 + 4)
                        for pc in range(2)
                    ]

                def out_proj(qq):
                    for f in oproj_fillers(qq):
                        f()

                # Input streams in first-use order on the one DMA queue:
                # xtq/xtk feed the first scores — interleaved by s-column so
                # the first projection groups start early; wv next (V input
                # slices are DMA'd per-group inside v_proj); wo last.
                xtq_c = load_xt(xtq)
                xtk_c = load_xt(xtk)
                nc.sync.dma_start(
                    wv_sb[:].rearrange("p (kc d) -> p kc d", d=DL),
                    wvt.ap().rearrange("(kc p) d -> p kc d", p=128),
                )
                nc.sync.dma_start(
                    wo_sb[:].rearrange("p (t o) -> p t o", o=D),
                    wot.ap().rearrange("(t p) o -> p t o", p=128),
                )

                # ---- pair-outer schedule.  Per pair: 4 ACT-bound attention
                # q-blocks; the NEXT pair's Q/K projections are emitted
                # after them (lower scheduler priority) so they fill the PE
                # idle inside the exp-paced stream.  attention(0,0) is
                # chunk-interleaved with the V projection (its first
                # consumer); out-projection rides the last pair's stretch.
                # first pair's projections interleaved q/k per s-column so
                # the first scores (needing q col-block 0 + k col-block 0)
                # start after two groups, not eight
                for sc in range(SC4):
                    qk_group(xtq_c, wq_sb, qt_sb, bq_sb, 0, sc)
                    qk_group(xtk_c, wk_sb, kt_sb, bk_sb, 0, sc)
                # Filler work (next pair's Q/K groups; previous q-block's
                # out-projection pieces) is threaded one-piece-per-chunk
                # into the exp-paced attention stream so it lands in PE idle
                # windows instead of serializing at block boundaries.
                for pr in range(DC):
                    for qq in range(4):
                        inter = v_proj if (pr == 0 and qq == 0) else None
                        fill = None
                        if pr < DC - 1 and qq == 3:
                            nxt = pr + 1
                            fill = [
                                (lambda w, x, o, b, sc: lambda: qk_group(x, w, o, b, nxt, sc))(
                                    w, x, o, b, sc
                                )
                                for sc in range(SC4)
                                for w, x, o, b in (
                                    (wq_sb, xtq_c, qt_sb, bq_sb),
                                    (wk_sb, xtk_c, kt_sb, bk_sb),
                                )
                            ]
                        elif pr == DC - 1 and qq >= 1:
                            fill = oproj_fillers(qq - 1)
                        attention(qq, pr, interleave=inter, fillers=fill)
                out_proj(3)

    nc.compile()
    return nc


def get_program():
    global _PROGRAM
    if _PROGRAM is None:
        _PROGRAM = _build_program()
    return _PROGRAM


def make_in_maps(query, key, value, Wq, bq, Wk, bk, Wv, bv, Wo, bo):
    """Per-core input dicts. Core c: batch c//2, head-group c%2."""
    query = np.asarray(query, np.float32)
    key = np.asarray(key, np.float32)
    value = np.asarray(value, np.float32)
    xt = {}
    for b in range(B):
        xt[b] = (
            np.ascontiguousarray(query[b].T.astype(np.float16)),
            np.ascontiguousarray(key[b].T.astype(np.float16)),
            np.ascontiguousarray(value[b].T.astype(np.float16)),
        )
    wslices = {}
    for hg in range(2):
        sl = slice(hg * DL, (hg + 1) * DL)
        wslices[hg] = dict(
            wqt=np.ascontiguousarray(np.asarray(Wq, np.float32)[sl, :].T.astype(np.float16)),
            wkt=np.ascontiguousarray(np.asarray(Wk, np.float32)[sl, :].T.astype(np.float16)),
            wvt=np.ascontiguousarray(np.asarray(Wv, np.float32)[sl, :].T.astype(np.float16)),
            wot=np.ascontiguousarray(np.asarray(Wo, np.float32)[:, sl].T.astype(np.float16)),
            bq=np.ascontiguousarray(np.asarray(bq, np.float32)[sl]),
            bk=np.ascontiguousarray(np.asarray(bk, np.float32)[sl]),
        )
    in_maps = []
    for c in range(NCORES):
        b, hg = c // 2, c % 2
        m = dict(xtq=xt[b][0], xtk=xt[b][1], xtv=xt[b][2])
        m.update(wslices[hg])
        in_maps.append(m)
    return in_maps


def combine_outputs(results, Wo, bo, bv):
    """Sum the two head-group partials per batch + host-side bias constant."""
    Wo = np.asarray(Wo, np.float32)
    bo = np.asarray(bo, np.float32)
    bv = np.asarray(bv, np.float32)
    const = bv @ Wo.T + bo  # [D]
    out = np.empty((B, S, D), np.float32)
    for b in range(B):
        out[b] = results[2 * b]["y"] + results[2 * b + 1]["y"] + const
    return out


def kernel(query, key, value, Wq, bq, Wk, bk, Wv, bv, Wo, bo):
    from concourse.bass_utils import run_bass_kernel_spmd

    nc = get_program()
    in_maps = make_in_maps(query, key, value, Wq, bq, Wk, bk, Wv, bv, Wo, bo)
    res = run_bass_kernel_spmd(nc, in_maps, core_ids=list(range(NCORES)))
    return combine_outputs(res.results, Wo, bo, bv)


# revision 44
# speedup vs baseline: 1.0332x; 1.0105x over previous
"""Multi-head attention Trainium2 Bass kernel.

Problem: B=4, S=2048, D=1024, H=16 heads (head_dim 64).
  q = (query @ Wq.T + bq).astype(f16); k, v likewise
  energy = einsum('bhqd,bhkd', q, k) / sqrt(64)   (f16)
  attn = softmax(energy, -1)                       (f16)
  x = einsum('bhqk,bhkd', attn, v).astype(f32)
  out = x @ Wo.T + bo                              (f32)

Sharding (8 cores): core c handles batch b = c//2 and head-group hg = c%2
(8 heads = 512 of the 1024 hidden dims).  QKV projections are column-split,
out-projection is row-split; the two partial outputs per batch are summed on
the host.  Biases: bq/bk are added on-chip (per-partition bias on the ACT
eviction); bv/bo contribute `bv_local @ WoT_local + bo` — a constant row
(softmax rows sum to 1) added on the host.

On-chip dataflow per core (all f16 matmul inputs, f32 PSUM):
  1. QT = WqT_loc.T @ XTq  -> [512, 2048] (d_local on partitions), same KT.
     V = XTv.T @ WvT_loc   -> [2048, 512] (s on partitions), stored per-head
     with an appended ones column (V_aug [128, 8*65]).
  2. Per head pair (row-tiled PE, head0 partitions 0:64 / head1 64:128) and
     q-half of 1024: for each k-chunk of 128:
       ST[k,q] scores (transposed layout), one ACT exp over [128, 2048]
       (scale=1/8 fused, no max subtraction - energies are ~N(0,1), exp fits
       f16 comfortably), AV matmul with ones-augmented V accumulating
       O_unnorm.T [65, 1024]; row 64 = softmax denominator.
  3. Normalize: reciprocal of denom row, gpsimd partition-broadcast,
     DVE multiply -> OT f16 (pair-packed [128, 2048] per d-chunk; odd head
     routed through a base-0 tmp tile + SBUF DMA to partitions 64:128).
  4. Out-projection: Y[q,1024] = sum_t OT_t.T @ WoT_t, f32 out.
"""

import os

import numpy as np

B, S, D, H = 4, 2048, 1024, 16
HD = 64
NCORES = 8
DL = 512  # d_local per core
HL = 8  # local heads per core
KC = 8  # contraction chunks (D / 128) for projections
DC = 4  # d_local chunks of 128
SC4 = 4  # S chunks of 512
SC16 = 16  # S chunks of 128
VW = HD + 1  # per-head V width incl. ones column (65)

_PROGRAM = None


def _build_program():
    import concourse.mybir as mybir
    import concourse.tile as tile
    from concourse import bacc

    f16 = mybir.dt.float16
    f32 = mybir.dt.float32
    ACT = mybir.ActivationFunctionType

    nc = bacc.Bacc("TRN2", target_bir_lowering=False, debug=False)

    xtq = nc.declare_dram_parameter("xtq", [D, S], f16, isOutput=False)
    xtk = nc.declare_dram_parameter("xtk", [D, S], f16, isOutput=False)
    xtv = nc.declare_dram_parameter("xtv", [D, S], f16, isOutput=False)
    wqt = nc.declare_dram_parameter("wqt", [D, DL], f16, isOutput=False)
    wkt = nc.declare_dram_parameter("wkt", [D, DL], f16, isOutput=False)
    wvt = nc.declare_dram_parameter("wvt", [D, DL], f16, isOutput=False)
    wot = nc.declare_dram_parameter("wot", [DL, D], f16, isOutput=False)
    bq = nc.declare_dram_parameter("bq", [DL], f32, isOutput=False)
    bk = nc.declare_dram_parameter("bk", [DL], f32, isOutput=False)
    y = nc.declare_dram_parameter("y", [S, D], f32, isOutput=True)

    with tile.TileContext(nc) as tc:
        # ---- persistent SBUF pools ----
        with (
            tc.tile_pool(name="wpool", bufs=1) as wpool,
            tc.tile_pool(name="bpool", bufs=1) as bpool,
            tc.tile_pool(name="qkv_sb", bufs=1) as qkv_sb,
            tc.tile_pool(name="ot_sb", bufs=1) as ot_pool,
        ):
            # weights: wx_sb[p, kc*512 + d] = WxT[kc*128 + p, d].  DMA order
            # matters (one HW queue): issue in first-use order — wv before
            # the xtv stream, wq/wk next, wo (used last) at the end.
            wq_sb = wpool.tile([128, KC * DL], f16, name="wq_sb")
            wk_sb = wpool.tile([128, KC * DL], f16, name="wk_sb")
            wv_sb = wpool.tile([128, KC * DL], f16, name="wv_sb")
            # wo_sb[p, t*1024 + o] = WoT[t*128 + p, o]
            wo_sb = wpool.tile([128, DC * D], f16, name="wo_sb")
            nc.sync.dma_start(
                wq_sb[:].rearrange("p (kc d) -> p kc d", d=DL),
                wqt.ap().rearrange("(kc p) d -> p kc d", p=128),
            )
            nc.sync.dma_start(
                wk_sb[:].rearrange("p (kc d) -> p kc d", d=DL),
                wkt.ap().rearrange("(kc p) d -> p kc d", p=128),
            )
            # biases as [128, DC] (per-partition scalars per d-chunk)
            bq_sb = bpool.tile([128, DC], f32, name="bq_sb")
            bk_sb = bpool.tile([128, DC], f32, name="bk_sb")
            nc.sync.dma_start(bq_sb[:], bq.ap().rearrange("(t p) -> p t", p=128))
            nc.sync.dma_start(bk_sb[:], bk.ap().rearrange("(t p) -> p t", p=128))

            # persistent activations
            qt_sb = [qkv_sb.tile([128, S], f16, name=f"qt{t}") for t in range(DC)]
            kt_sb = [qkv_sb.tile([128, S], f16, name=f"kt{t}") for t in range(DC)]
            v_sb = [qkv_sb.tile([128, HL * VW], f16, name=f"v{sc}") for sc in range(SC16)]
            ot_sb = [ot_pool.tile([128, S], f16, name=f"ot{t}") for t in range(DC)]

            # One PSUM pool for the whole kernel, three tags:
            #   stq: [128, 1024] f32, 2 slots (scores double-buffer)
            #   av:  [65, 512] f32, 2 slots (AV accumulator pair)
            #   ps:  [128, 512] f32, 2 slots (QKV + out-projection
            #        accumulators — never busy at the same time)
            # 2*2 + 2*1 + 2*1 = 8 banks.  Separate av/ps tags let attention
            # start while the K projection is still draining.
            with (
                tc.tile_pool(name="psum", bufs=1, space="PSUM") as psum,
                tc.tile_pool(name="xt_pool", bufs=16) as xt_pool,
                tc.tile_pool(name="vsl_pool", bufs=3) as vsl_pool,
                tc.tile_pool(name="e_pool", bufs=5) as e_pool,
                tc.tile_pool(name="n_pool", bufs=1) as n_pool,
                tc.tile_pool(name="y_pool", bufs=2) as y_pool,
            ):

                def load_xt(x_dram, engine=None):
                    # per-contraction-chunk tiles: xt_c[kc][p, s] = XT[kc*128+p, s]
                    engine = engine or nc.sync
                    xt_c = []
                    for kc in range(KC):
                        xt_t = xt_pool.tile([128, S], f16, name="xt", tag="xt")
                        engine.dma_start(
                            xt_t[:], x_dram.ap()[kc * 128 : (kc + 1) * 128, :]
                        )
                        xt_c.append(xt_t)
                    return xt_c

                def qk_group(xt_c, w_sb, out_tiles, b_ap, dc, sc):
                    # one QT/KT projection group: out [d_local(part), 512 s]
                    ps = psum.tile([128, 512], f32, name="ps", tag="ps", bufs=2)
                    for kc in range(KC):
                        nc.tensor.matmul(
                            ps[:],
                            lhsT=w_sb[:, kc * DL + dc * 128 : kc * DL + dc * 128 + 128],
                            rhs=xt_c[kc][:, sc * 512 : (sc + 1) * 512],
                            start=(kc == 0),
                            stop=(kc == KC - 1),
                        )
                    # eviction + per-partition bias on DVE (keeps ACT free
                    # for attention exp)
                    nc.vector.tensor_scalar_add(
                        out_tiles[dc][:, sc * 512 : (sc + 1) * 512],
                        ps[:],
                        b_ap[:, dc : dc + 1],
                    )

                def qk_proj(xt_c, w_sb, out_tiles, b_ap, dc):
                    for sc in range(SC4):
                        qk_group(xt_c, w_sb, out_tiles, b_ap, dc, sc)

                def v_proj(sc):
                    # own sliced input tile vt[p, kc*128+si] = XTv[kc*128+p,
                    # sc*128+si] — keeps xtv out of the xt slot ring (a
                    # shared ring deadlocks: xtv loads would wait on slots
                    # that only free after the last Q/K projection, which is
                    # scheduled after attention that needs V)
                    vt = vsl_pool.tile([128, KC * 128], f16, name="vt", tag="vt")
                    nc.sync.dma_start(
                        vt[:].rearrange("p (kc si) -> p kc si", si=128),
                        xtv.ap().rearrange("(kc p) s -> p kc s", p=128)[
                            :, :, sc * 128 : (sc + 1) * 128
                        ],
                    )
                    ps = psum.tile([128, 512], f32, name="ps", tag="ps", bufs=2)
                    for kc in range(KC):
                        nc.tensor.matmul(
                            ps[:],
                            lhsT=vt[:, kc * 128 : (kc + 1) * 128],
                            rhs=wv_sb[:, kc * DL : (kc + 1) * DL],
                            start=(kc == 0),
                            stop=(kc == KC - 1),
                        )
                    v3 = v_sb[sc][:].rearrange("p (h x) -> p h x", x=VW)
                    nc.vector.tensor_copy(
                        v3[:, :, 0:HD], ps[:].rearrange("p (h x) -> p h x", x=HD)
                    )
                    nc.vector.memset(v3[:, :, HD : HD + 1], 1.0)

                def attention(qq, pr, interleave=None, fillers=None):
                    q0 = qq * 512
                    h0, h1 = 2 * pr, 2 * pr + 1
                    av0 = psum.tile([VW, 512], f32, name="av", tag="av", bufs=2)
                    av1 = psum.tile([VW, 512], f32, name="av", tag="av", bufs=2)
                    for kc in range(SC16):
                        if interleave is not None:
                            interleave(kc)
                        st = psum.tile([128, 1024], f32, name="st", tag="stq", bufs=2)
                        nc.tensor.matmul(
                            st[:, 0:512],
                            lhsT=kt_sb[pr][0:64, kc * 128 : (kc + 1) * 128],
                            rhs=qt_sb[pr][0:64, q0 : q0 + 512],
                            start=True,
                            stop=True,
                        )
                        nc.tensor.matmul(
                            st[:, 512:1024],
                            lhsT=kt_sb[pr][64:128, kc * 128 : (kc + 1) * 128],
                            rhs=qt_sb[pr][64:128, q0 : q0 + 512],
                            start=True,
                            stop=True,
                        )
                        e = e_pool.tile([128, 1024], f16, name="e", tag="e")
                        nc.scalar.activation(e[:], st[:], ACT.Exp, scale=0.125)
                        nc.tensor.matmul(
                            av0[:],
                            lhsT=v_sb[kc][:, h0 * VW : (h0 + 1) * VW],
                            rhs=e[:, 0:512],
                            start=(kc == 0),
                            stop=(kc == SC16 - 1),
                            skip_group_check=True,
                        )
                        nc.tensor.matmul(
                            av1[:],
                            lhsT=v_sb[kc][:, h1 * VW : (h1 + 1) * VW],
                            rhs=e[:, 512:1024],
                            start=(kc == 0),
                            stop=(kc == SC16 - 1),
                            skip_group_check=True,
                        )
                        if fillers:
                            fillers.pop(0)()
                    # Evict AV PSUM -> SBUF immediately (frees the banks for
                    # the next pair; normalization then runs out of SBUF off
                    # the PE critical path).
                    od0 = n_pool.tile([VW, 512], f32, name="od0", tag="od0")
                    od1 = n_pool.tile([VW, 512], f32, name="od1", tag="od1")
                    nc.vector.tensor_copy(od0[:], av0[:])
                    nc.vector.tensor_copy(od1[:], av1[:])
                    # normalize: O.T[hd, q] * (1 / denom[q]).  Denom rows sit
                    # at partition 64; DVE keeps partition bases aligned and
                    # partition_broadcast only reads base-0 offset-0 sources
                    # on HW: SBUF DMA down to p0 -> reciprocal -> gpsimd
                    # broadcast.
                    dd = n_pool.tile([1, 1024], f32, name="dd", tag="dd")
                    nc.sync.dma_start(dd[:, 0:512], od0[HD : HD + 1, :])
                    nc.sync.dma_start(dd[:, 512:1024], od1[HD : HD + 1, :])
                    r0 = n_pool.tile([1, 512], f32, name="r0", tag="r0")
                    r1 = n_pool.tile([1, 512], f32, name="r1", tag="r1")
                    nc.vector.reciprocal_approx_fast(r0[:], dd[:, 0:512])
                    nc.vector.reciprocal_approx_fast(r1[:], dd[:, 512:1024])
                    bc_a = n_pool.tile([64, 512], f32, name="bc_a", tag="bc_a")
                    bc_b = n_pool.tile([64, 512], f32, name="bc_b", tag="bc_b")
                    nc.gpsimd.partition_broadcast(bc_a[:], r0[:])
                    nc.gpsimd.partition_broadcast(bc_b[:], r1[:])
                    cols = slice(q0, q0 + 512)
                    nc.vector.tensor_mul(ot_sb[pr][0:64, cols], od0[0:64, :], bc_a[:])
                    tmp = n_pool.tile([64, 512], f16, name="tmp", tag="tmp")
                    nc.vector.tensor_mul(tmp[:], od1[0:64, :], bc_b[:])
                    nc.sync.dma_start(ot_sb[pr][64:128, cols], tmp[:])

                def oproj_piece(mc, pc, state):
                    # one 512-wide accumulation group of the out-projection
                    # for q-chunk mc; pc==1 finishes the chunk (evict + DMA)
                    if pc == 0:
                        state[mc] = y_pool.tile([128, 1024], f32, name="yt", tag="yt")
                    yt = state[mc]
                    pso = psum.tile([128, 512], f32, name="pso", tag="ps", bufs=2)
                    for t in range(DC):
                        nc.tensor.matmul(
                            pso[:],
                            lhsT=ot_sb[t][:, mc * 128 : (mc + 1) * 128],
                            rhs=wo_sb[:, t * D + pc * 512 : t * D + (pc + 1) * 512],
                            start=(t == 0),
                            stop=(t == DC - 1),
                            skip_group_check=True,
                        )
                    nc.vector.tensor_copy(yt[:, pc * 512 : (pc + 1) * 512], pso[:])
                    if pc == 1:
                        nc.sync.dma_start(y.ap()[mc * 128 : (mc + 1) * 128, :], yt[:])

                def oproj_fillers(qq):
                    state = {}
                    return [
                        (lambda mc=mc, pc=pc: oproj_piece(mc, pc, state))
                        for mc in range(qq * 4, qq * 4 + 4)
                        for pc in range(2)
                    ]

                def out_proj(qq):
                    for f in oproj_fillers(qq):
                        f()

                # Input streams in first-use order on the one DMA queue:
                # xtq/xtk feed the first scores — interleaved by s-column so
                # the first projection groups start early; wv next (V input
                # slices are DMA'd per-group inside v_proj); wo last.
                xtq_c = load_xt(xtq)
                xtk_c = load_xt(xtk)
                nc.sync.dma_start(
                    wv_sb[:].rearrange("p (kc d) -> p kc d", d=DL),
                    wvt.ap().rearrange("(kc p) d -> p kc d", p=128),
                )
                nc.sync.dma_start(
                    wo_sb[:].rearrange("p (t o) -> p t o", o=D),
                    wot.ap().rearrange("(t p) o -> p t o", p=128),
                )

                # ---- pair-outer schedule.  Per pair: 4 ACT-bound attention
                # q-blocks; the NEXT pair's Q/K projections are emitted
                # after them (lower scheduler priority) so they fill the PE
                # idle inside the exp-paced stream.  attention(0,0) is
                # chunk-interleaved with the V projection (its first
                # consumer); out-projection rides the last pair's stretch.
                # first pair's projections interleaved q/k per s-column so
                # the first scores (needing q col-block 0 + k col-block 0)
                # start after two groups, not eight
                for sc in range(SC4):
                    qk_group(xtq_c, wq_sb, qt_sb, bq_sb, 0, sc)
                    qk_group(xtk_c, wk_sb, kt_sb, bk_sb, 0, sc)
                # Filler work (next pair's Q/K groups; previous q-block's
                # out-projection pieces) is threaded one-piece-per-chunk
                # into the exp-paced attention stream so it lands in PE idle
                # windows instead of serializing at block boundaries.
                for pr in range(DC):
                    for qq in range(4):
                        inter = v_proj if (pr == 0 and qq == 0) else None
                        fill = None
                        if pr < DC - 1 and qq == 3:
                            nxt = pr + 1
                            fill = [
                                (lambda w, x, o, b, sc: lambda: qk_group(x, w, o, b, nxt, sc))(
                                    w, x, o, b, sc
                                )
                                for sc in range(SC4)
                                for w, x, o, b in (
                                    (wq_sb, xtq_c, qt_sb, bq_sb),
                                    (wk_sb, xtk_c, kt_sb, bk_sb),
                                )
                            ]
                        elif pr == DC - 1 and qq >= 1:
                            fill = oproj_fillers(qq - 1)
                        attention(qq, pr, interleave=inter, fillers=fill)
                out_proj(3)

    nc.compile()
    return nc


def get_program():
    global _PROGRAM
    if _PROGRAM is None:
        _PROGRAM = _build_program()
    return _PROGRAM


def make_in_maps(query, key, value, Wq, bq, Wk, bk, Wv, bv, Wo, bo):
    """Per-core input dicts. Core c: batch c//2, head-group c%2."""
    query = np.asarray(query, np.float32)
    key = np.asarray(key, np.float32)
    value = np.asarray(value, np.float32)
    xt = {}
    for b in range(B):
        xt[b] = (
            np.ascontiguousarray(query[b].T.astype(np.float16)),
            np.ascontiguousarray(key[b].T.astype(np.float16)),
            np.ascontiguousarray(value[b].T.astype(np.float16)),
        )
    wslices = {}
    for hg in range(2):
        sl = slice(hg * DL, (hg + 1) * DL)
        wslices[hg] = dict(
            wqt=np.ascontiguousarray(np.asarray(Wq, np.float32)[sl, :].T.astype(np.float16)),
            wkt=np.ascontiguousarray(np.asarray(Wk, np.float32)[sl, :].T.astype(np.float16)),
            wvt=np.ascontiguousarray(np.asarray(Wv, np.float32)[sl, :].T.astype(np.float16)),
            wot=np.ascontiguousarray(np.asarray(Wo, np.float32)[:, sl].T.astype(np.float16)),
            bq=np.ascontiguousarray(np.asarray(bq, np.float32)[sl]),
            bk=np.ascontiguousarray(np.asarray(bk, np.float32)[sl]),
        )
    in_maps = []
    for c in range(NCORES):
        b, hg = c // 2, c % 2
        m = dict(xtq=xt[b][0], xtk=xt[b][1], xtv=xt[b][2])
        m.update(wslices[hg])
        in_maps.append(m)
    return in_maps


def combine_outputs(results, Wo, bo, bv):
    """Sum the two head-group partials per batch + host-side bias constant."""
    Wo = np.asarray(Wo, np.float32)
    bo = np.asarray(bo, np.float32)
    bv = np.asarray(bv, np.float32)
    const = bv @ Wo.T + bo  # [D]
    out = np.empty((B, S, D), np.float32)
    for b in range(B):
        out[b] = results[2 * b]["y"] + results[2 * b + 1]["y"] + const
    return out


def kernel(query, key, value, Wq, bq, Wk, bk, Wv, bv, Wo, bo):
    from concourse.bass_utils import run_bass_kernel_spmd

    nc = get_program()
    in_maps = make_in_maps(query, key, value, Wq, bq, Wk, bk, Wv, bv, Wo, bo)
    res = run_bass_kernel_spmd(nc, in_maps, core_ids=list(range(NCORES)))
    return combine_outputs(res.results, Wo, bo, bv)


# revision 45
# speedup vs baseline: 1.0410x; 1.0076x over previous
"""Multi-head attention Trainium2 Bass kernel.

Problem: B=4, S=2048, D=1024, H=16 heads (head_dim 64).
  q = (query @ Wq.T + bq).astype(f16); k, v likewise
  energy = einsum('bhqd,bhkd', q, k) / sqrt(64)   (f16)
  attn = softmax(energy, -1)                       (f16)
  x = einsum('bhqk,bhkd', attn, v).astype(f32)
  out = x @ Wo.T + bo                              (f32)

Sharding (8 cores): core c handles batch b = c//2 and head-group hg = c%2
(8 heads = 512 of the 1024 hidden dims).  QKV projections are column-split,
out-projection is row-split; the two partial outputs per batch are summed on
the host.  Biases: bq/bk are added on-chip (per-partition bias on the ACT
eviction); bv/bo contribute `bv_local @ WoT_local + bo` — a constant row
(softmax rows sum to 1) added on the host.

On-chip dataflow per core (all f16 matmul inputs, f32 PSUM):
  1. QT = WqT_loc.T @ XTq  -> [512, 2048] (d_local on partitions), same KT.
     V = XTv.T @ WvT_loc   -> [2048, 512] (s on partitions), stored per-head
     with an appended ones column (V_aug [128, 8*65]).
  2. Per head pair (row-tiled PE, head0 partitions 0:64 / head1 64:128) and
     q-half of 1024: for each k-chunk of 128:
       ST[k,q] scores (transposed layout), one ACT exp over [128, 2048]
       (scale=1/8 fused, no max subtraction - energies are ~N(0,1), exp fits
       f16 comfortably), AV matmul with ones-augmented V accumulating
       O_unnorm.T [65, 1024]; row 64 = softmax denominator.
  3. Normalize: reciprocal of denom row, gpsimd partition-broadcast,
     DVE multiply -> OT f16 (pair-packed [128, 2048] per d-chunk; odd head
     routed through a base-0 tmp tile + SBUF DMA to partitions 64:128).
  4. Out-projection: Y[q,1024] = sum_t OT_t.T @ WoT_t, f32 out.
"""

import os

import numpy as np

B, S, D, H = 4, 2048, 1024, 16
HD = 64
NCORES = 8
DL = 512  # d_local per core
HL = 8  # local heads per core
KC = 8  # contraction chunks (D / 128) for projections
DC = 4  # d_local chunks of 128
SC4 = 4  # S chunks of 512
SC16 = 16  # S chunks of 128
VW = HD + 1  # per-head V width incl. ones column (65)

_PROGRAM = None


def _build_program():
    import concourse.mybir as mybir
    import concourse.tile as tile
    from concourse import bacc

    f16 = mybir.dt.float16
    f32 = mybir.dt.float32
    ACT = mybir.ActivationFunctionType

    nc = bacc.Bacc("TRN2", target_bir_lowering=False, debug=False)

    xtq = nc.declare_dram_parameter("xtq", [D, S], f16, isOutput=False)
    xtk = nc.declare_dram_parameter("xtk", [D, S], f16, isOutput=False)
    xtv = nc.declare_dram_parameter("xtv", [D, S], f16, isOutput=False)
    wqt = nc.declare_dram_parameter("wqt", [D, DL], f16, isOutput=False)
    wkt = nc.declare_dram_parameter("wkt", [D, DL], f16, isOutput=False)
    wvt = nc.declare_dram_parameter("wvt", [D, DL], f16, isOutput=False)
    wot = nc.declare_dram_parameter("wot", [DL, D], f16, isOutput=False)
    bq = nc.declare_dram_parameter("bq", [DL], f32, isOutput=False)
    bk = nc.declare_dram_parameter("bk", [DL], f32, isOutput=False)
    y = nc.declare_dram_parameter("y", [S, D], f32, isOutput=True)

    with tile.TileContext(nc) as tc:
        # ---- persistent SBUF pools ----
        with (
            tc.tile_pool(name="wpool", bufs=1) as wpool,
            tc.tile_pool(name="bpool", bufs=1) as bpool,
            tc.tile_pool(name="qkv_sb", bufs=1) as qkv_sb,
            tc.tile_pool(name="ot_sb", bufs=1) as ot_pool,
        ):
            # weights: wx_sb[p, kc*512 + d] = WxT[kc*128 + p, d].  DMA order
            # matters (one HW queue): issue in first-use order — wv before
            # the xtv stream, wq/wk next, wo (used last) at the end.
            wq_sb = wpool.tile([128, KC * DL], f16, name="wq_sb")
            wk_sb = wpool.tile([128, KC * DL], f16, name="wk_sb")
            wv_sb = wpool.tile([128, KC * DL], f16, name="wv_sb")
            # wo_sb[p, t*1024 + o] = WoT[t*128 + p, o]
            wo_sb = wpool.tile([128, DC * D], f16, name="wo_sb")
            nc.sync.dma_start(
                wq_sb[:].rearrange("p (kc d) -> p kc d", d=DL),
                wqt.ap().rearrange("(kc p) d -> p kc d", p=128),
            )
            nc.sync.dma_start(
                wk_sb[:].rearrange("p (kc d) -> p kc d", d=DL),
                wkt.ap().rearrange("(kc p) d -> p kc d", p=128),
            )
            # biases as [128, DC] (per-partition scalars per d-chunk)
            bq_sb = bpool.tile([128, DC], f32, name="bq_sb")
            bk_sb = bpool.tile([128, DC], f32, name="bk_sb")
            nc.sync.dma_start(bq_sb[:], bq.ap().rearrange("(t p) -> p t", p=128))
            nc.sync.dma_start(bk_sb[:], bk.ap().rearrange("(t p) -> p t", p=128))

            # persistent activations
            qt_sb = [qkv_sb.tile([128, S], f16, name=f"qt{t}") for t in range(DC)]
            kt_sb = [qkv_sb.tile([128, S], f16, name=f"kt{t}") for t in range(DC)]
            v_sb = [qkv_sb.tile([128, HL * VW], f16, name=f"v{sc}") for sc in range(SC16)]
            ot_sb = [ot_pool.tile([128, S], f16, name=f"ot{t}") for t in range(DC)]

            # One PSUM pool for the whole kernel, three tags:
            #   stq: [128, 1024] f32, 2 slots (scores double-buffer)
            #   av:  [65, 512] f32, 2 slots (AV accumulator pair)
            #   ps:  [128, 512] f32, 2 slots (QKV + out-projection
            #        accumulators — never busy at the same time)
            # 2*2 + 2*1 + 2*1 = 8 banks.  Separate av/ps tags let attention
            # start while the K projection is still draining.
            with (
                tc.tile_pool(name="psum", bufs=1, space="PSUM") as psum,
                tc.tile_pool(name="xt_pool", bufs=16) as xt_pool,
                tc.tile_pool(name="vsl_pool", bufs=3) as vsl_pool,
                tc.tile_pool(name="e_pool", bufs=5) as e_pool,
                tc.tile_pool(name="n_pool", bufs=1) as n_pool,
                tc.tile_pool(name="y_pool", bufs=2) as y_pool,
            ):

                def load_xt(x_dram, engine=None):
                    # per-contraction-chunk tiles: xt_c[kc][p, s] = XT[kc*128+p, s]
                    engine = engine or nc.sync
                    xt_c = []
                    for kc in range(KC):
                        xt_t = xt_pool.tile([128, S], f16, name="xt", tag="xt")
                        engine.dma_start(
                            xt_t[:], x_dram.ap()[kc * 128 : (kc + 1) * 128, :]
                        )
                        xt_c.append(xt_t)
                    return xt_c

                def qk_group(xt_c, w_sb, out_tiles, b_ap, dc, sc):
                    # one QT/KT projection group: out [d_local(part), 512 s]
                    ps = psum.tile([128, 512], f32, name="ps", tag="ps", bufs=2)
                    for kc in range(KC):
                        nc.tensor.matmul(
                            ps[:],
                            lhsT=w_sb[:, kc * DL + dc * 128 : kc * DL + dc * 128 + 128],
                            rhs=xt_c[kc][:, sc * 512 : (sc + 1) * 512],
                            start=(kc == 0),
                            stop=(kc == KC - 1),
                        )
                    # eviction + per-partition bias on DVE (keeps ACT free
                    # for attention exp)
                    nc.vector.tensor_scalar_add(
                        out_tiles[dc][:, sc * 512 : (sc + 1) * 512],
                        ps[:],
                        b_ap[:, dc : dc + 1],
                    )

                def qk_proj(xt_c, w_sb, out_tiles, b_ap, dc):
                    for sc in range(SC4):
                        qk_group(xt_c, w_sb, out_tiles, b_ap, dc, sc)

                def v_proj(sc):
                    # own sliced input tile vt[p, kc*128+si] = XTv[kc*128+p,
                    # sc*128+si] — keeps xtv out of the xt slot ring (a
                    # shared ring deadlocks: xtv loads would wait on slots
                    # that only free after the last Q/K projection, which is
                    # scheduled after attention that needs V)
                    vt = vsl_pool.tile([128, KC * 128], f16, name="vt", tag="vt")
                    nc.sync.dma_start(
                        vt[:].rearrange("p (kc si) -> p kc si", si=128),
                        xtv.ap().rearrange("(kc p) s -> p kc s", p=128)[
                            :, :, sc * 128 : (sc + 1) * 128
                        ],
                    )
                    ps = psum.tile([128, 512], f32, name="ps", tag="ps", bufs=2)
                    for kc in range(KC):
                        nc.tensor.matmul(
                            ps[:],
                            lhsT=vt[:, kc * 128 : (kc + 1) * 128],
                            rhs=wv_sb[:, kc * DL : (kc + 1) * DL],
                            start=(kc == 0),
                            stop=(kc == KC - 1),
                        )
                    v3 = v_sb[sc][:].rearrange("p (h x) -> p h x", x=VW)
                    nc.vector.tensor_copy(
                        v3[:, :, 0:HD], ps[:].rearrange("p (h x) -> p h x", x=HD)
                    )
                    nc.vector.memset(v3[:, :, HD : HD + 1], 1.0)

                def attention(qq, pr, interleave=None, fillers=None):
                    q0 = qq * 512
                    h0, h1 = 2 * pr, 2 * pr + 1
                    av0 = psum.tile([VW, 512], f32, name="av", tag="av", bufs=2)
                    av1 = psum.tile([VW, 512], f32, name="av", tag="av", bufs=2)
                    for kc in range(SC16):
                        if interleave is not None:
                            interleave(kc)
                        st = psum.tile([128, 1024], f32, name="st", tag="stq", bufs=2)
                        nc.tensor.matmul(
                            st[:, 0:512],
                            lhsT=kt_sb[pr][0:64, kc * 128 : (kc + 1) * 128],
                            rhs=qt_sb[pr][0:64, q0 : q0 + 512],
                            start=True,
                            stop=True,
                        )
                        nc.tensor.matmul(
                            st[:, 512:1024],
                            lhsT=kt_sb[pr][64:128, kc * 128 : (kc + 1) * 128],
                            rhs=qt_sb[pr][64:128, q0 : q0 + 512],
                            start=True,
                            stop=True,
                        )
                        e = e_pool.tile([128, 1024], f16, name="e", tag="e")
                        nc.scalar.activation(e[:], st[:], ACT.Exp, scale=0.125)
                        nc.tensor.matmul(
                            av0[:],
                            lhsT=v_sb[kc][:, h0 * VW : (h0 + 1) * VW],
                            rhs=e[:, 0:512],
                            start=(kc == 0),
                            stop=(kc == SC16 - 1),
                            skip_group_check=True,
                        )
                        nc.tensor.matmul(
                            av1[:],
                            lhsT=v_sb[kc][:, h1 * VW : (h1 + 1) * VW],
                            rhs=e[:, 512:1024],
                            start=(kc == 0),
                            stop=(kc == SC16 - 1),
                            skip_group_check=True,
                        )
                        if fillers:
                            fillers.pop(0)()
                    # Evict AV PSUM -> SBUF immediately (frees the banks for
                    # the next pair; normalization then runs out of SBUF off
                    # the PE critical path).
                    od0 = n_pool.tile([VW, 512], f32, name="od0", tag="od0")
                    od1 = n_pool.tile([VW, 512], f32, name="od1", tag="od1")
                    nc.vector.tensor_copy(od0[:], av0[:])
                    nc.vector.tensor_copy(od1[:], av1[:])
                    # normalize: O.T[hd, q] * (1 / denom[q]).  Denom rows sit
                    # at partition 64; DVE keeps partition bases aligned and
                    # partition_broadcast only reads base-0 offset-0 sources
                    # on HW: SBUF DMA down to p0 -> reciprocal -> gpsimd
                    # broadcast.
                    dd = n_pool.tile([1, 1024], f32, name="dd", tag="dd")
                    nc.sync.dma_start(dd[:, 0:512], od0[HD : HD + 1, :])
                    nc.sync.dma_start(dd[:, 512:1024], od1[HD : HD + 1, :])
                    r0 = n_pool.tile([1, 512], f32, name="r0", tag="r0")
                    r1 = n_pool.tile([1, 512], f32, name="r1", tag="r1")
                    nc.vector.reciprocal_approx_fast(r0[:], dd[:, 0:512])
                    nc.vector.reciprocal_approx_fast(r1[:], dd[:, 512:1024])
                    bc_a = n_pool.tile([64, 512], f32, name="bc_a", tag="bc_a")
                    bc_b = n_pool.tile([64, 512], f32, name="bc_b", tag="bc_b")
                    nc.gpsimd.partition_broadcast(bc_a[:], r0[:])
                    nc.gpsimd.partition_broadcast(bc_b[:], r1[:])
                    cols = slice(q0, q0 + 512)
                    nc.vector.tensor_mul(ot_sb[pr][0:64, cols], od0[0:64, :], bc_a[:])
                    tmp = n_pool.tile([64, 512], f16, name="tmp", tag="tmp")
                    nc.vector.tensor_mul(tmp[:], od1[0:64, :], bc_b[:])
                    nc.sync.dma_start(ot_sb[pr][64:128, cols], tmp[:])

                def oproj_piece(mc, pc, state):
                    # one 512-wide accumulation group of the out-projection
                    # for q-chunk mc; pc==1 finishes the chunk (evict + DMA)
                    if pc == 0:
                        state[mc] = y_pool.tile([128, 1024], f32, name="yt", tag="yt")
                    yt = state[mc]
                    pso = psum.tile([128, 512], f32, name="pso", tag="ps", bufs=2)
                    for t in range(DC):
                        nc.tensor.matmul(
                            pso[:],
                            lhsT=ot_sb[t][:, mc * 128 : (mc + 1) * 128],
                            rhs=wo_sb[:, t * D + pc * 512 : t * D + (pc + 1) * 512],
                            start=(t == 0),
                            stop=(t == DC - 1),
                            skip_group_check=True,
                        )
                    nc.vector.tensor_copy(yt[:, pc * 512 : (pc + 1) * 512], pso[:])
                    if pc == 1:
                        nc.sync.dma_start(y.ap()[mc * 128 : (mc + 1) * 128, :], yt[:])

                def oproj_fillers(qq):
                    state = {}
                    return [
                        (lambda mc=mc, pc=pc: oproj_piece(mc, pc, state))
                        for mc in range(qq * 4, qq * 4 + 4)
                        for pc in range(2)
                    ]

                def out_proj(qq):
                    for f in oproj_fillers(qq):
                        f()

                # Input streams in first-use order on the one DMA queue:
                # xtq/xtk feed the first scores — interleaved by s-column so
                # the first projection groups start early; wv next (V input
                # slices are DMA'd per-group inside v_proj); wo last.
                xtq_c = load_xt(xtq)
                xtk_c = load_xt(xtk)
                nc.sync.dma_start(
                    wv_sb[:].rearrange("p (kc d) -> p kc d", d=DL),
                    wvt.ap().rearrange("(kc p) d -> p kc d", p=128),
                )
                nc.sync.dma_start(
                    wo_sb[:].rearrange("p (t o) -> p t o", o=D),
                    wot.ap().rearrange("(t p) o -> p t o", p=128),
                )

                # ---- pair-outer schedule.  Per pair: 4 ACT-bound attention
                # q-blocks; the NEXT pair's Q/K projections are emitted
                # after them (lower scheduler priority) so they fill the PE
                # idle inside the exp-paced stream.  attention(0,0) is
                # chunk-interleaved with the V projection (its first
                # consumer); out-projection rides the last pair's stretch.
                # first pair's projections interleaved q/k per s-column so
                # the first scores (needing q col-block 0 + k col-block 0)
                # start after two groups, not eight
                for sc in range(SC4):
                    qk_group(xtq_c, wq_sb, qt_sb, bq_sb, 0, sc)
                    qk_group(xtk_c, wk_sb, kt_sb, bk_sb, 0, sc)
                # Next pair's Q/K projections are emitted after each pair's
                # four q-blocks (lower scheduler priority -> they fill PE
                # idle inside the exp-paced stream); out-projection rides
                # the last pair's stretch.
                for pr in range(DC):
                    for qq in range(4):
                        inter = v_proj if (pr == 0 and qq == 0) else None
                        attention(qq, pr, interleave=inter)
                        if pr == DC - 1:
                            out_proj(qq)
                    if pr < DC - 1:
                        qk_proj(xtq_c, wq_sb, qt_sb, bq_sb, pr + 1)
                        qk_proj(xtk_c, wk_sb, kt_sb, bk_sb, pr + 1)

    nc.compile()
    return nc


def get_program():
    global _PROGRAM
    if _PROGRAM is None:
        _PROGRAM = _build_program()
    return _PROGRAM


def make_in_maps(query, key, value, Wq, bq, Wk, bk, Wv, bv, Wo, bo):
    """Per-core input dicts. Core c: batch c//2, head-group c%2."""
    query = np.asarray(query, np.float32)
    key = np.asarray(key, np.float32)
    value = np.asarray(value, np.float32)
    xt = {}
    for b in range(B):
        xt[b] = (
            np.ascontiguousarray(query[b].T.astype(np.float16)),
            np.ascontiguousarray(key[b].T.astype(np.float16)),
            np.ascontiguousarray(value[b].T.astype(np.float16)),
        )
    wslices = {}
    for hg in range(2):
        sl = slice(hg * DL, (hg + 1) * DL)
        wslices[hg] = dict(
            wqt=np.ascontiguousarray(np.asarray(Wq, np.float32)[sl, :].T.astype(np.float16)),
            wkt=np.ascontiguousarray(np.asarray(Wk, np.float32)[sl, :].T.astype(np.float16)),
            wvt=np.ascontiguousarray(np.asarray(Wv, np.float32)[sl, :].T.astype(np.float16)),
            wot=np.ascontiguousarray(np.asarray(Wo, np.float32)[:, sl].T.astype(np.float16)),
            bq=np.ascontiguousarray(np.asarray(bq, np.float32)[sl]),
            bk=np.ascontiguousarray(np.asarray(bk, np.float32)[sl]),
        )
    in_maps = []
    for c in range(NCORES):
        b, hg = c // 2, c % 2
        m = dict(xtq=xt[b][0], xtk=xt[b][1], xtv=xt[b][2])
        m.update(wslices[hg])
        in_maps.append(m)
    return in_maps


def combine_outputs(results, Wo, bo, bv):
    """Sum the two head-group partials per batch + host-side bias constant."""
    Wo = np.asarray(Wo, np.float32)
    bo = np.asarray(bo, np.float32)
    bv = np.asarray(bv, np.float32)
    const = bv @ Wo.T + bo  # [D]
    out = np.empty((B, S, D), np.float32)
    for b in range(B):
        out[b] = results[2 * b]["y"] + results[2 * b + 1]["y"] + const
    return out


def kernel(query, key, value, Wq, bq, Wk, bk, Wv, bv, Wo, bo):
    from concourse.bass_utils import run_bass_kernel_spmd

    nc = get_program()
    in_maps = make_in_maps(query, key, value, Wq, bq, Wk, bk, Wv, bv, Wo, bo)
    res = run_bass_kernel_spmd(nc, in_maps, core_ids=list(range(NCORES)))
    return combine_outputs(res.results, Wo, bo, bv)
